# revision 11
# baseline (speedup 1.0000x reference)
"""Position Attention Module (DANet) on 8 Trainium2 NeuronCores.

Reference computation (per batch b of 4):
  xf = x[b] : [C=512, N=4096]
  q = Wq@xf + bq : [64, N];  k = Wk@xf + bk : [64, N];  v = Wv@xf + bv : [512, N]
  scores[i,j] = q[:,i].k[:,j];  attn = softmax_j(scores)
  out[c,i] = alpha * sum_j v[c,j] attn[i,j]

Sharding: 2 cores per batch, each core owns half the query rows (i), full k.
Per-core x is pre-rolled on host so the owned i-half is always columns 0:2048.

Device design (v9):
  - alpha folded into Wv/bv on the host.
  - 16-bit tensor datapath everywhere (power: f32r-dense kernels DVFS-throttle
    the PE clock). q/k path in fp16 (same 11-bit mantissa as f32r; all values
    ~N(0,1), far from fp16 range limits); AV accumulation and output
    projection in bf16 (exp values need fp32 exponent range). Single-pass
    K=64 scores. Measured error 5.4e-3 vs 2e-2 tolerance.
  - exp without max-subtraction: scores ~ N(0,64); |s|max ~ 56 << 88.
  - V path reassociated: out = Wv@(x@attnT) + bv, so the 512x512 projection
    runs on the 2048 attention-averaged columns instead of the 4096 v
    columns, and v never materializes. Host sends x fp16 (for q/k) and a
    j-tile-paired bf16 transpose xt (for the AV lhsT).
  - softmax denominator: DVE-accumulated exp sums (f32r); ones[128,128] f32r
    matmul broadcasts the partition-sum to all lanes; DVE reciprocal.
    Normalization is applied at the *output* eviction (DVE mul) so the
    attention PSUM accumulators are freed by plain copies right after the
    j-loop.
  - in-order engine queues: scores emitted two j-steps ahead of the AV
    accumulation; the previous i-tile's output projection interleaves into
    the early j-steps of the next i-tile; the final i-tile's projection
    cycles through the idle scores-PSUM ring to avoid single-bank
    serialization.
  - DMA: two HWDGE queues (qSyIO: x + weights, qAct: xt) run concurrently;
    tiles are [128, 1024] so every packet moves 2KB per partition; wvt/bv
    stream after x (needed only once the first i-tile finishes).
"""
import numpy as np
import ml_dtypes


B, C, HW = 4, 512, 4096
CQ = 64
NCORES = 8
IH = HW // 2          # 2048 query rows per core
ITILE = 512           # i-tile (psum free dim)
NITILES = IH // ITILE # 4
JT = 128              # j-tile (contraction chunk for attn / scores lhsT cols)
NJT = HW // JT        # 32
JB = 1024             # j-block for projections / DMA tiles
NJB = HW // JB        # 4
NCC = C // 128        # 4 contraction chunks of 128 over C

_cache = {}


def _build():
    import concourse.bacc as bacc
    import concourse.tile as tile
    import concourse.mybir as mybir
    from concourse.bass_utils import run_bass_kernel_spmd

    f32 = mybir.dt.float32
    f32r = mybir.dt.float32r
    bf16 = mybir.dt.bfloat16
    fp16 = mybir.dt.float16
    AF = mybir.ActivationFunctionType

    nc = bacc.Bacc("TRN2", target_bir_lowering=False, debug=False)

    x_d = nc.dram_tensor("x", [C, HW], fp16, kind="ExternalInput")
    # xt packed in pairs of j-tiles: tile g row p holds xT[2g*128+p, :] and
    # xT[(2g+1)*128+p, :] side by side (2KB contiguous per partition row).
    xt_d = nc.dram_tensor("xt", [128, NJT * C], bf16, kind="ExternalInput")
    wqt_d = nc.dram_tensor("wqt", [C, CQ], fp16, kind="ExternalInput")
    wkt_d = nc.dram_tensor("wkt", [C, CQ], fp16, kind="ExternalInput")
    wvt_d = nc.dram_tensor("wvt", [C, C], bf16, kind="ExternalInput")
    bq_d = nc.dram_tensor("bq", [CQ, 1], f32, kind="ExternalInput")
    bk_d = nc.dram_tensor("bk", [CQ, 1], f32, kind="ExternalInput")
    bv_d = nc.dram_tensor("bv", [C, 1], f32, kind="ExternalInput")
    out_d = nc.dram_tensor("out", [C, IH], f32, kind="ExternalOutput")

    with tile.TileContext(nc) as tc:
        with (
            tc.tile_pool(name="const", bufs=1) as cpool,
            tc.tile_pool(name="kq", bufs=1) as kqpool,
            tc.tile_pool(name="xt", bufs=1) as xtpool,
        ):
            # --- early constants: q/k weights + biases (small, needed first) ---
            wqt = [cpool.tile([128, CQ], fp16, tag=f"wqt{i}", name=f"wqt{i}") for i in range(NCC)]
            wkt = [cpool.tile([128, CQ], fp16, tag=f"wkt{i}", name=f"wkt{i}") for i in range(NCC)]
            wvt = [cpool.tile([128, C], bf16, tag=f"wvt{i}", name=f"wvt{i}") for i in range(NCC)]
            for cc in range(NCC):
                sl = slice(cc * 128, (cc + 1) * 128)
                nc.sync.dma_start(wkt[cc][:], wkt_d[sl, :])
                nc.sync.dma_start(wqt[cc][:], wqt_d[sl, :])
            bq_c = cpool.tile([CQ, 1], f32, tag="bqc")
            bk_c = cpool.tile([CQ, 1], f32, tag="bkc")
            nc.sync.dma_start(bq_c[:], bq_d[:])
            nc.sync.dma_start(bk_c[:], bk_d[:])
            bv_c = cpool.tile([128, NCC], f32, tag="bvc")
            ones_f = cpool.tile([128, 128], f32, tag="onesf")
            nc.vector.memset(ones_f[:], 1.0)
            ones_sq = cpool.tile([128, 128], f32r, tag="onessq")  # sum+bcast lhsT
            nc.vector.tensor_copy(ones_sq[:], ones_f[:])

            # resident activations: k/q [CQ, *] fp16, xT pair-tiles [128, 2C] bf16
            k_sb = kqpool.tile([CQ, HW], fp16, tag="ksb")
            q_sb = kqpool.tile([CQ, IH], fp16, tag="qsb")
            xt2 = [xtpool.tile([128, 2 * C], bf16, tag=f"xt{g}", name=f"xt{g}") for g in range(NJT // 2)]
            for g in range(NJT // 2):
                nc.scalar.dma_start(xt2[g][:], xt_d[:, g * 2 * C:(g + 1) * 2 * C])

            def xt_slice(j, cc):
                return xt2[j // 2][:, (j % 2) * C + cc * 128:(j % 2) * C + (cc + 1) * 128]

            # ---------------- q/k projections ----------------
            with (
                tc.tile_pool(name="xin", bufs=8) as xpool,
                tc.tile_pool(name="pkq", bufs=3, space="PSUM") as pkq,
            ):
                for jb in range(NJB):
                    bsl = slice(jb * JB, (jb + 1) * JB)
                    xt = []
                    for cc in range(NCC):
                        csl = slice(cc * 128, (cc + 1) * 128)
                        t = xpool.tile([128, JB], fp16, tag="x", name=f"x{jb}_{cc}")
                        if jb == 0:
                            nc.sync.dma_start(t[:, 0:512], x_d[csl, 0:512])
                        else:
                            nc.sync.dma_start(t[:], x_d[csl, bsl])
                        xt.append(t)
                    if jb == 0:
                        for cc in range(NCC):
                            csl = slice(cc * 128, (cc + 1) * 128)
                            nc.sync.dma_start(xt[cc][:, 512:1024], x_d[csl, 512:1024])
                    for h in range(2):
                        jsl = slice(jb * JB + h * 512, jb * JB + (h + 1) * 512)
                        hsl = slice(h * 512, (h + 1) * 512)
                        kp = pkq.tile([CQ, 512], f32, tag="kqp")
                        for cc in range(NCC):
                            nc.tensor.matmul(kp[:], wkt[cc][:], xt[cc][:, hsl],
                                             start=(cc == 0), stop=(cc == NCC - 1))
                        nc.scalar.activation(k_sb[:, jsl], kp[:], AF.Identity, bias=bk_c[:])
                        if jb < NJB // 2:
                            qp = pkq.tile([CQ, 512], f32, tag="kqp")
                            for cc in range(NCC):
                                nc.tensor.matmul(qp[:], wqt[cc][:], xt[cc][:, hsl],
                                                 start=(cc == 0), stop=(cc == NCC - 1))
                            nc.scalar.activation(q_sb[:, jsl], qp[:], AF.Identity, bias=bq_c[:])

            # late weights: needed only from the first output projection on
            for cc in range(NCC):
                nc.scalar.dma_start(wvt[cc][:], wvt_d[cc * 128:(cc + 1) * 128, :])
                nc.scalar.dma_start(bv_c[:, cc:cc + 1], bv_d[cc * 128:(cc + 1) * 128, :])

            # ---------------- attention + output projection ----------------
            with (
                tc.tile_pool(name="expp", bufs=6) as epool,
                tc.tile_pool(name="dnm", bufs=2) as dpool,
                tc.tile_pool(name="ysb", bufs=8) as ypool,
                tc.tile_pool(name="ost", bufs=3) as opool,
                tc.tile_pool(name="rows", bufs=2) as rpool,
                tc.tile_pool(name="psc", bufs=4, space="PSUM") as psc,
                tc.tile_pool(name="py", bufs=4, space="PSUM") as py,
            ):
                prev = None  # (it, ysb[4], recipB) of the previous i-tile

                def emit_outproj(itp, co, ysb, recipB):
                    ipsl = slice(itp * ITILE, (itp + 1) * ITILE)
                    op = psc.tile([128, ITILE], f32, tag="sc", name=f"op{itp}_{co}")
                    for ci in range(NCC):
                        nc.tensor.matmul(
                            op[:], wvt[ci][:, co * 128:(co + 1) * 128], ysb[ci][:],
                            start=(ci == 0), stop=(ci == NCC - 1))
                    ou = opool.tile([128, ITILE], f32, tag="ot", name=f"ou{itp}_{co}")
                    nc.vector.tensor_mul(ou[:], op[:], recipB[:])
                    ob = opool.tile([128, ITILE], f32, tag="ob", name=f"ob{itp}_{co}")
                    nc.scalar.activation(ob[:], ou[:], AF.Identity, bias=bv_c[:, co:co + 1])
                    nc.sync.dma_start(out_d[co * 128:(co + 1) * 128, ipsl], ob[:])

                for it in range(NITILES):
                    isl = slice(it * ITILE, (it + 1) * ITILE)
                    yps = [py.tile([128, ITILE], f32, tag="yp", name=f"yp{it}_{i}") for i in range(NCC)]
                    dnm = dpool.tile([128, ITILE], f32r, tag="dn")
                    ets = {}

                    def emit_scores(j):
                        jsl = slice(j * JT, (j + 1) * JT)
                        sp = psc.tile([JT, ITILE], f32, tag="sc", name=f"sc{it}_{j}")
                        nc.tensor.matmul(sp[:], k_sb[:, jsl], q_sb[:, isl],
                                         start=True, stop=True)
                        et = epool.tile([JT, ITILE], bf16, tag="exp", name=f"et{it}_{j}")
                        nc.scalar.activation(et[:], sp[:], AF.Exp)
                        ets[j] = et

                    emit_scores(0)
                    emit_scores(1)
                    for j in range(NJT):
                        if j + 2 < NJT:
                            emit_scores(j + 2)
                        et = ets.pop(j)
                        if j == 0:
                            nc.vector.tensor_copy(dnm[:], et[:])
                        else:
                            nc.vector.tensor_add(dnm[:], dnm[:], et[:])
                        for cc in range(NCC):
                            nc.tensor.matmul(
                                yps[cc][:], xt_slice(j, cc), et[:],
                                start=(j == 0), stop=(j == NJT - 1))
                        if prev is not None and j in (2, 4, 6, 8):
                            emit_outproj(prev[0], (j - 2) // 2, prev[1], prev[2])

                    # denomB = column-sums of dnm broadcast to all 128 partitions
                    dB = psc.tile([128, ITILE], f32, tag="sc", name=f"dB{it}")
                    nc.tensor.matmul(dB[:], ones_sq[:], dnm[:], start=True, stop=True)
                    recipB = rpool.tile([128, ITILE], f32, tag="recipB")
                    nc.vector.reciprocal_approx_fast(out=recipB[:], in_=dB[:])
                    # free the accumulators: raw (unnormalized) bf16 copies
                    ysb = [ypool.tile([128, ITILE], bf16, tag="y", name=f"y{it}_{i}") for i in range(NCC)]
                    for cc in range(NCC):
                        nc.vector.tensor_copy(ysb[cc][:], yps[cc][:])
                    prev = (it, ysb, recipB)

                for co in range(NCC):
                    emit_outproj(prev[0], co, prev[1], prev[2])

    nc.compile()
    return nc, run_bass_kernel_spmd


def kernel(x, Wq, bq, Wk, bk, Wv, bv, alpha, trace=False, trace_kwargs=None):
    if "nc" not in _cache:
        _cache["nc"] = _build()
    nc, run_spmd = _cache["nc"]

    x = np.ascontiguousarray(np.asarray(x, dtype=np.float32)).reshape(B, C, HW)
    a = float(np.asarray(alpha, np.float32).reshape(-1)[0])
    wqt = np.ascontiguousarray(np.asarray(Wq, np.float32).T.astype(np.float16))
    wkt = np.ascontiguousarray(np.asarray(Wk, np.float32).T.astype(np.float16))
    wvt = np.ascontiguousarray((np.asarray(Wv, np.float32).T * a).astype(ml_dtypes.bfloat16))
    bq = np.asarray(bq, np.float32).reshape(CQ, 1)
    bk = np.asarray(bk, np.float32).reshape(CQ, 1)
    bv = (np.asarray(bv, np.float32) * a).reshape(C, 1)

    in_maps = []
    for core in range(NCORES):
        b, ih = core // 2, core % 2
        xb = x[b]
        if ih:
            xb = np.ascontiguousarray(np.concatenate([xb[:, IH:], xb[:, :IH]], axis=1))
        xtp = np.ascontiguousarray(
            xb.T.astype(ml_dtypes.bfloat16).reshape(NJT, 128, C)
            .transpose(1, 0, 2).reshape(128, NJT * C))
        in_maps.append({"x": xb.astype(np.float16), "xt": xtp,
                        "wqt": wqt, "wkt": wkt, "wvt": wvt,
                        "bq": bq, "bk": bk, "bv": bv})

    kwargs = {}
    if trace:
        kwargs["trace"] = True
        kwargs.update(trace_kwargs or {})
    res = run_spmd(nc, in_maps, list(range(NCORES)), **kwargs)

    out = np.empty((B, C, HW), dtype=np.float32)
    for core in range(NCORES):
        b, ih = core // 2, core % 2
        out[b][:, ih * IH:(ih + 1) * IH] = res.results[core]["out"]
    if trace:
        return out.reshape(B, C, 64, 64), res
    return out.reshape(B, C, 64, 64)


# revision 12
# speedup vs baseline: 1.0270x; 1.0270x over previous
"""Position Attention Module (DANet) on 8 Trainium2 NeuronCores.

Reference computation (per batch b of 4):
  xf = x[b] : [C=512, N=4096]
  q = Wq@xf + bq : [64, N];  k = Wk@xf + bk : [64, N];  v = Wv@xf + bv : [512, N]
  scores[i,j] = q[:,i].k[:,j];  attn = softmax_j(scores)
  out[c,i] = alpha * sum_j v[c,j] attn[i,j]

Sharding: 2 cores per batch, each core owns half the query rows (i), full k.
Per-core x is pre-rolled on host so the owned i-half is always columns 0:2048.

Device design (v9):
  - alpha folded into Wv/bv on the host.
  - 16-bit tensor datapath everywhere (power: f32r-dense kernels DVFS-throttle
    the PE clock). q/k path in fp16 (same 11-bit mantissa as f32r; all values
    ~N(0,1), far from fp16 range limits); AV accumulation and output
    projection in bf16 (exp values need fp32 exponent range). Single-pass
    K=64 scores. Measured error 5.4e-3 vs 2e-2 tolerance.
  - exp without max-subtraction: scores ~ N(0,64); |s|max ~ 56 << 88.
  - V path reassociated: out = Wv@(x@attnT) + bv, so the 512x512 projection
    runs on the 2048 attention-averaged columns instead of the 4096 v
    columns, and v never materializes. Host sends x fp16 (for q/k) and a
    j-tile-paired bf16 transpose xt (for the AV lhsT).
  - softmax denominator: DVE-accumulated exp sums (f32r); ones[128,128] f32r
    matmul broadcasts the partition-sum to all lanes; DVE reciprocal.
    Normalization is applied at the *output* eviction (DVE mul) so the
    attention PSUM accumulators are freed by plain copies right after the
    j-loop.
  - in-order engine queues: scores emitted two j-steps ahead of the AV
    accumulation; the previous i-tile's output projection interleaves into
    the early j-steps of the next i-tile; the final i-tile's projection
    cycles through the idle scores-PSUM ring to avoid single-bank
    serialization.
  - DMA: two HWDGE queues (qSyIO: x + weights, qAct: xt) run concurrently;
    tiles are [128, 1024] so every packet moves 2KB per partition; wvt/bv
    stream after x (needed only once the first i-tile finishes).
"""
import numpy as np
import ml_dtypes


B, C, HW = 4, 512, 4096
CQ = 64
NCORES = 8
IH = HW // 2          # 2048 query rows per core
ITILE = 512           # i-tile (psum free dim)
NITILES = IH // ITILE # 4
JT = 128              # j-tile (contraction chunk for attn / scores lhsT cols)
NJT = HW // JT        # 32
JB = 1024             # j-block for projections / DMA tiles
NJB = HW // JB        # 4
NCC = C // 128        # 4 contraction chunks of 128 over C

_cache = {}


def _build():
    import concourse.bacc as bacc
    import concourse.tile as tile
    import concourse.mybir as mybir
    from concourse.bass_utils import run_bass_kernel_spmd

    f32 = mybir.dt.float32
    f32r = mybir.dt.float32r
    bf16 = mybir.dt.bfloat16
    fp16 = mybir.dt.float16
    AF = mybir.ActivationFunctionType

    nc = bacc.Bacc("TRN2", target_bir_lowering=False, debug=False)

    x_d = nc.dram_tensor("x", [C, HW], fp16, kind="ExternalInput")
    # xt packed in pairs of j-tiles: tile g row p holds xT[2g*128+p, :] and
    # xT[(2g+1)*128+p, :] side by side (2KB contiguous per partition row).
    xt_d = nc.dram_tensor("xt", [128, NJT * C], bf16, kind="ExternalInput")
    wqkt_d = nc.dram_tensor("wqkt", [C, 2 * CQ], fp16, kind="ExternalInput")
    wvt_d = nc.dram_tensor("wvt", [C, C], bf16, kind="ExternalInput")
    bq_d = nc.dram_tensor("bq", [CQ, 1], f32, kind="ExternalInput")
    bk_d = nc.dram_tensor("bk", [CQ, 1], f32, kind="ExternalInput")
    bv_d = nc.dram_tensor("bv", [C, 1], f32, kind="ExternalInput")
    out_d = nc.dram_tensor("out", [C, IH], f32, kind="ExternalOutput")

    with tile.TileContext(nc) as tc:
        with (
            tc.tile_pool(name="const", bufs=1) as cpool,
            tc.tile_pool(name="kq", bufs=1) as kqpool,
            tc.tile_pool(name="xt", bufs=1) as xtpool,
        ):
            # --- early constants: q/k weights + biases (small, needed first) ---
            wqkt = [cpool.tile([128, 2 * CQ], fp16, tag=f"wqkt{i}", name=f"wqkt{i}") for i in range(NCC)]
            wvt = [cpool.tile([128, C], bf16, tag=f"wvt{i}", name=f"wvt{i}") for i in range(NCC)]
            for cc in range(NCC):
                sl = slice(cc * 128, (cc + 1) * 128)
                nc.sync.dma_start(wqkt[cc][:], wqkt_d[sl, :])
            bq_c = cpool.tile([CQ, 1], f32, tag="bqc")
            bk_c = cpool.tile([CQ, 1], f32, tag="bkc")
            nc.sync.dma_start(bq_c[:], bq_d[:])
            nc.sync.dma_start(bk_c[:], bk_d[:])
            bv_c = cpool.tile([128, NCC], f32, tag="bvc")
            ones_f = cpool.tile([128, 128], f32, tag="onesf")
            nc.vector.memset(ones_f[:], 1.0)
            ones_sq = cpool.tile([128, 128], f32r, tag="onessq")  # sum+bcast lhsT
            nc.vector.tensor_copy(ones_sq[:], ones_f[:])

            # resident activations: k/q [CQ, *] fp16, xT pair-tiles [128, 2C] bf16
            k_sb = kqpool.tile([CQ, HW], fp16, tag="ksb")
            q_sb = kqpool.tile([CQ, IH], fp16, tag="qsb")
            xt2 = [xtpool.tile([128, 2 * C], bf16, tag=f"xt{g}", name=f"xt{g}") for g in range(NJT // 2)]
            for g in range(NJT // 2):
                nc.scalar.dma_start(xt2[g][:], xt_d[:, g * 2 * C:(g + 1) * 2 * C])

            def xt_slice(j, cc):
                return xt2[j // 2][:, (j % 2) * C + cc * 128:(j % 2) * C + (cc + 1) * 128]

            # ---------------- q/k projections ----------------
            with (
                tc.tile_pool(name="xin", bufs=8) as xpool,
                tc.tile_pool(name="pkq", bufs=3, space="PSUM") as pkq,
            ):
                for jb in range(NJB):
                    bsl = slice(jb * JB, (jb + 1) * JB)
                    xt = []
                    for cc in range(NCC):
                        csl = slice(cc * 128, (cc + 1) * 128)
                        t = xpool.tile([128, JB], fp16, tag="x", name=f"x{jb}_{cc}")
                        if jb == 0:
                            nc.sync.dma_start(t[:, 0:512], x_d[csl, 0:512])
                        else:
                            nc.sync.dma_start(t[:], x_d[csl, bsl])
                        xt.append(t)
                    if jb == 0:
                        for cc in range(NCC):
                            csl = slice(cc * 128, (cc + 1) * 128)
                            nc.sync.dma_start(xt[cc][:, 512:1024], x_d[csl, 512:1024])
                    for h in range(2):
                        jsl = slice(jb * JB + h * 512, jb * JB + (h + 1) * 512)
                        hsl = slice(h * 512, (h + 1) * 512)
                        nq = 2 * CQ if jb < NJB // 2 else CQ
                        kp = pkq.tile([128, 512], f32, tag="kqp")
                        for cc in range(NCC):
                            nc.tensor.matmul(kp[0:nq, :], wqkt[cc][:, 0:nq], xt[cc][:, hsl],
                                             start=(cc == 0), stop=(cc == NCC - 1))
                        nc.scalar.activation(k_sb[:, jsl], kp[0:CQ, :], AF.Identity, bias=bk_c[:])
                        if jb < NJB // 2:
                            nc.scalar.activation(q_sb[:, jsl], kp[CQ:2 * CQ, :], AF.Identity, bias=bq_c[:])

            # late weights: needed only from the first output projection on
            for cc in range(NCC):
                nc.scalar.dma_start(wvt[cc][:], wvt_d[cc * 128:(cc + 1) * 128, :])
                nc.scalar.dma_start(bv_c[:, cc:cc + 1], bv_d[cc * 128:(cc + 1) * 128, :])

            # ---------------- attention + output projection ----------------
            with (
                tc.tile_pool(name="expp", bufs=6) as epool,
                tc.tile_pool(name="dnm", bufs=2) as dpool,
                tc.tile_pool(name="ysb", bufs=8) as ypool,
                tc.tile_pool(name="ost", bufs=3) as opool,
                tc.tile_pool(name="rows", bufs=2) as rpool,
                tc.tile_pool(name="psc", bufs=4, space="PSUM") as psc,
                tc.tile_pool(name="py", bufs=4, space="PSUM") as py,
            ):
                prev = None  # (it, ysb[4], recipB) of the previous i-tile

                def emit_outproj(itp, co, ysb, recipB):
                    ipsl = slice(itp * ITILE, (itp + 1) * ITILE)
                    op = psc.tile([128, ITILE], f32, tag="sc", name=f"op{itp}_{co}")
                    for ci in range(NCC):
                        nc.tensor.matmul(
                            op[:], wvt[ci][:, co * 128:(co + 1) * 128], ysb[ci][:],
                            start=(ci == 0), stop=(ci == NCC - 1))
                    ou = opool.tile([128, ITILE], f32, tag="ot", name=f"ou{itp}_{co}")
                    nc.vector.tensor_mul(ou[:], op[:], recipB[:])
                    ob = opool.tile([128, ITILE], f32, tag="ob", name=f"ob{itp}_{co}")
                    nc.scalar.activation(ob[:], ou[:], AF.Identity, bias=bv_c[:, co:co + 1])
                    nc.sync.dma_start(out_d[co * 128:(co + 1) * 128, ipsl], ob[:])

                for it in range(NITILES):
                    isl = slice(it * ITILE, (it + 1) * ITILE)
                    yps = [py.tile([128, ITILE], f32, tag="yp", name=f"yp{it}_{i}") for i in range(NCC)]
                    dnm = dpool.tile([128, ITILE], f32r, tag="dn")
                    ets = {}

                    def emit_scores(j):
                        jsl = slice(j * JT, (j + 1) * JT)
                        sp = psc.tile([JT, ITILE], f32, tag="sc", name=f"sc{it}_{j}")
                        nc.tensor.matmul(sp[:], k_sb[:, jsl], q_sb[:, isl],
                                         start=True, stop=True)
                        et = epool.tile([JT, ITILE], bf16, tag="exp", name=f"et{it}_{j}")
                        nc.scalar.activation(et[:], sp[:], AF.Exp)
                        ets[j] = et

                    emit_scores(0)
                    emit_scores(1)
                    emit_scores(2)
                    for j in range(NJT):
                        if j + 3 < NJT:
                            emit_scores(j + 3)
                        et = ets.pop(j)
                        if j == 0:
                            nc.vector.tensor_copy(dnm[:], et[:])
                        else:
                            nc.vector.tensor_add(dnm[:], dnm[:], et[:])
                        for cc in range(NCC):
                            nc.tensor.matmul(
                                yps[cc][:], xt_slice(j, cc), et[:],
                                start=(j == 0), stop=(j == NJT - 1))
                        if prev is not None and j in (2, 4, 6, 8):
                            emit_outproj(prev[0], (j - 2) // 2, prev[1], prev[2])

                    # denomB = column-sums of dnm broadcast to all 128 partitions
                    dB = psc.tile([128, ITILE], f32, tag="sc", name=f"dB{it}")
                    nc.tensor.matmul(dB[:], ones_sq[:], dnm[:], start=True, stop=True)
                    recipB = rpool.tile([128, ITILE], f32, tag="recipB")
                    nc.vector.reciprocal_approx_fast(out=recipB[:], in_=dB[:])
                    # free the accumulators: raw (unnormalized) bf16 copies
                    ysb = [ypool.tile([128, ITILE], bf16, tag="y", name=f"y{it}_{i}") for i in range(NCC)]
                    for cc in range(NCC):
                        nc.vector.tensor_copy(ysb[cc][:], yps[cc][:])
                    prev = (it, ysb, recipB)

                for co in range(NCC):
                    emit_outproj(prev[0], co, prev[1], prev[2])

    nc.compile()
    return nc, run_bass_kernel_spmd


def kernel(x, Wq, bq, Wk, bk, Wv, bv, alpha, trace=False, trace_kwargs=None):
    if "nc" not in _cache:
        _cache["nc"] = _build()
    nc, run_spmd = _cache["nc"]

    x = np.ascontiguousarray(np.asarray(x, dtype=np.float32)).reshape(B, C, HW)
    a = float(np.asarray(alpha, np.float32).reshape(-1)[0])
    wqkt = np.ascontiguousarray(np.concatenate(
        [np.asarray(Wk, np.float32).T, np.asarray(Wq, np.float32).T],
        axis=1).astype(np.float16))
    wvt = np.ascontiguousarray((np.asarray(Wv, np.float32).T * a).astype(ml_dtypes.bfloat16))
    bq = np.asarray(bq, np.float32).reshape(CQ, 1)
    bk = np.asarray(bk, np.float32).reshape(CQ, 1)
    bv = (np.asarray(bv, np.float32) * a).reshape(C, 1)

    in_maps = []
    for core in range(NCORES):
        b, ih = core // 2, core % 2
        xb = x[b]
        if ih:
            xb = np.ascontiguousarray(np.concatenate([xb[:, IH:], xb[:, :IH]], axis=1))
        xtp = np.ascontiguousarray(
            xb.T.astype(ml_dtypes.bfloat16).reshape(NJT, 128, C)
            .transpose(1, 0, 2).reshape(128, NJT * C))
        in_maps.append({"x": xb.astype(np.float16), "xt": xtp,
                        "wqkt": wqkt, "wvt": wvt,
                        "bq": bq, "bk": bk, "bv": bv})

    kwargs = {}
    if trace:
        kwargs["trace"] = True
        kwargs.update(trace_kwargs or {})
    res = run_spmd(nc, in_maps, list(range(NCORES)), **kwargs)

    out = np.empty((B, C, HW), dtype=np.float32)
    for core in range(NCORES):
        b, ih = core // 2, core % 2
        out[b][:, ih * IH:(ih + 1) * IH] = res.results[core]["out"]
    if trace:
        return out.reshape(B, C, 64, 64), res
    return out.reshape(B, C, 64, 64)
